# revision 3
# baseline (speedup 1.0000x reference)
"""v3 Bass/Trainium2 kernel for the 2-layer BiLSTM.

Design (per core, data-parallel batch slice B=16, 2 independent
direction-chains per layer-phase, staggered by the Tile scheduler):

 - JIT input-GEMM: every Sj=8 steps, per chain, the W_ih @ x contribution
   for the next 8 steps is matmul'd directly into the gates PSUM bank
   ([m][t8][b] layout, start=True); the per-step W_hh @ h matmuls then
   ACCUMULATE into their step slice (start=False).  No gx DRAM tensors,
   no separate GEMM phases, no gates add op.
 - tanh-free + mul-free cell: with sigmoid-only activations and host
   pre-scaling (g-gate rows x2; whole W_hh/W_ih1 x2 because h is stored
   halved) the cell is:
       t1q = (sg - 0.5) * si          (DVE scalar_tensor_tensor)
       t2  = sf * C_prev              (GPSIMD tensor_tensor)
       C   = 4*t1q + t2               (DVE scalar_tensor_tensor)
       sc  = sigmoid(C)               (ACT)
       h/2 = (sc - 0.5) * so          (GPSIMD scalar_tensor_tensor, fp16)
   where C = 2c.  Final outputs are doubled on the host.
 - per step per chain: 4 fp16 matmuls (+1 amortized JIT matmul), 2 ACT
   sigmoids, 2 DVE ops, 2 GPSIMD ops.
 - time reversal: host-reversed x for the L0 bwd chain; reversed-AP ring
   loads for the L1 bwd chain; reversed-AP block stores for bwd h.
"""

import numpy as np

import concourse.bass as bass
import concourse.bacc as bacc
import concourse.tile as tile
import concourse.mybir as mybir
from concourse import bass_utils

F32 = mybir.dt.float32
F16 = mybir.dt.float16
AF = mybir.ActivationFunctionType
OP = mybir.AluOpType

H = 100
NCORES = 8
BC = 16
SJ = 8            # steps per JIT input-GEMM group
REPEAT = 1        # workload repetitions (for slope timing)

_PERM = np.concatenate([np.arange(0, 100), np.arange(100, 200),
                        np.arange(300, 400), np.arange(200, 300)])


def build_program(T=1024, SB=32, SG=None):
    assert T % SB == 0 and SB % SJ == 0
    nc = bacc.Bacc("TRN2", target_bir_lowering=False, debug=False,
                   num_devices=NCORES)
    dram = {}

    def din(name, shape, dt=F32):
        dram[name] = nc.dram_tensor(name, shape, dt, kind="ExternalInput")

    def dout(name, shape, dt=F32):
        dram[name] = nc.dram_tensor(name, shape, dt, kind="ExternalOutput")

    def dint(name, shape, dt=F32):
        dram[name] = nc.dram_tensor(name, shape, dt, kind="Internal")

    din("xe", (H + 1, T, BC), F16)             # x + ones row (fwd)
    din("xer", (H + 1, T, BC), F16)            # time-reversed (bwd)
    for d in "fb":
        din(f"whh0{d}", (H, 4, 128), F16)
        din(f"whh1{d}", (H, 4, 128), F16)
        din(f"wih0{d}", (H + 1, 4, 128), F16)
        din(f"wih1a{d}", (H, 4, 128), F16)
        din(f"wih1b{d}", (H + 1, 4, 128), F16)
    dout("h1f", (H, T, BC), F16)
    dout("h1b", (H, T, BC), F16)
    dint("hf0", (H, T, BC), F16)
    dint("hb0e", (H + 1, T, BC), F16)

    with tile.TileContext(nc) as tc:
        _emit(tc, nc, dram, T, SB)
    return nc


def _emit(tc, nc, dram, T, SB):
    from contextlib import ExitStack
    ctx = ExitStack()
    wpool = ctx.enter_context(tc.tile_pool(name="weights", bufs=1))
    xpool = ctx.enter_context(tc.tile_pool(name="xring", bufs=3))
    gpsum = ctx.enter_context(tc.tile_pool(name="gates", bufs=2, space="PSUM"))
    hpool = ctx.enter_context(tc.tile_pool(name="hring", bufs=2))
    spool = ctx.enter_context(tc.tile_pool(name="cell", bufs=3))
    cpool = ctx.enter_context(tc.tile_pool(name="cstate", bufs=2))

    # ---- weights + constants ----------------------------------------
    w_sb = {}
    for name in ("whh0f", "whh0b", "whh1f", "whh1b"):
        t = wpool.tile([H, 4 * 128], F16, tag=name, name=name)
        nc.sync.dma_start(t[:].rearrange("p (m q) -> p m q", m=4),
                          dram[name].ap())
        w_sb[name] = t
    for name in ("wih0f", "wih0b"):
        t = wpool.tile([H + 1, 4 * 128], F16, tag=name, name=name)
        nc.sync.dma_start(t[:].rearrange("p (m q) -> p m q", m=4),
                          dram[name].ap())
        w_sb[name] = t
    for name in ("wih1af", "wih1ab"):
        t = wpool.tile([H, 4 * 128], F16, tag=name, name=name)
        nc.sync.dma_start(t[:].rearrange("p (m q) -> p m q", m=4),
                          dram[name].ap())
        w_sb[name] = t
    for name in ("wih1bf", "wih1bb"):
        t = wpool.tile([H + 1, 4 * 128], F16, tag=name, name=name)
        nc.sync.dma_start(t[:].rearrange("p (m q) -> p m q", m=4),
                          dram[name].ap())
        w_sb[name] = t

    zeroh = wpool.tile([H, BC], F16, tag="zeroh")
    nc.vector.memset(zeroh[:], 0.0)
    zeroc = wpool.tile([H, BC], F32, tag="zeroc")
    nc.vector.memset(zeroc[:], 0.0)
    ones16 = wpool.tile([1, 2048], F16, tag="ones16")
    nc.vector.memset(ones16[:], 1.0)
    hb0e = dram["hb0e"].ap()
    onesrow = hb0e[H:H + 1, :, :].rearrange("p t b -> p (t b)")
    for k in range(0, T * BC, 2048):
        w = min(2048, T * BC - k)
        nc.sync.dma_start(onesrow[:, k:k + w], ones16[:, 0:w])

    NG = SJ * BC                     # JIT group moving width (128)

    def recurrence(layer):
        nb = T // SB
        ng = T // SJ
        if layer == 0:
            houts = {"f": dram["hf0"].ap(), "b": dram["hb0e"].ap()[0:H, :, :]}
        else:
            houts = {"f": dram["h1f"].ap(), "b": dram["h1b"].ap()}
        st = {}
        for d in "fb":
            st[d] = dict(
                whh=w_sb[f"whh{layer}{d}"],
                hout=houts[d], h_prev=zeroh[:], c_prev=zeroc[:],
                rings={}, banks={}, R=None)

        def load_ring(d, b):
            """ring tiles for block b (processing order)."""
            c = st[d]
            rev = (d == "b")
            pblk = (nb - 1 - b) if rev else b
            if layer == 0:
                xa = xpool.tile([H + 1, SB * BC], F16, tag=f"xa{d}",
                                name=f"xa{d}")
                srcv = dram["xe" if d == "f" else "xer"].ap()
                nc.sync.dma_start(
                    xa[:].rearrange("p (t b) -> p t b", t=SB),
                    srcv[:, b * SB:(b + 1) * SB, :])
                c["rings"][b] = (xa, None)
            else:
                xa = xpool.tile([H, SB * BC], F16, tag=f"xa{d}", name=f"xa{d}")
                xb = xpool.tile([H + 1, SB * BC], F16, tag=f"xb{d}",
                                name=f"xb{d}")
                for ring, t_ in (("hf0", xa), ("hb0e", xb)):
                    srcv = dram[ring].ap()[:, pblk * SB:(pblk + 1) * SB, :]
                    if rev:
                        srcv = srcv[:, ::-1, :]
                    nc.sync.dma_start(
                        t_[:].rearrange("p (t b) -> p t b", t=SB), srcv)
                c["rings"][b] = (xa, xb)
            c["rings"].pop(b - 3, None)

        def jit_piece(d, g, k):
            """emit JIT matmul piece k for step-group g (if any)."""
            if g >= ng:
                return
            c = st[d]
            npiece = 4 if layer == 0 else 8
            if k >= npiece:
                return
            if k == 0:
                c["banks"][g] = gpsum.tile([128, 4 * NG], F32,
                                           tag=f"bank{d}", name=f"bank{d}")
                c["banks"].pop(g - 2, None)
            bank = c["banks"][g]
            blk_of_g = (g * SJ) // SB
            xa, xb = c["rings"][blk_of_g]
            off = (g * SJ) % SB
            mv = slice(off * BC, (off + SJ) * BC)
            if layer == 0:
                m = k
                nc.tensor.matmul(
                    bank[:, m * NG:(m + 1) * NG],
                    w_sb[f"wih0{d}"][:, m * 128:(m + 1) * 128],
                    xa[:, mv], start=(k == 0), stop=False,
                    skip_group_check=True)
            else:
                m, half = k % 4, k // 4
                w = w_sb[f"wih1a{d}"] if half == 0 else w_sb[f"wih1b{d}"]
                x = xa if half == 0 else xb
                nc.tensor.matmul(
                    bank[:, m * NG:(m + 1) * NG],
                    w[:, m * 128:(m + 1) * 128],
                    x[:, mv], start=(k == 0), stop=False,
                    skip_group_check=True)

        # prologue: ring block 0 (+1), JIT group 0
        for d in "fb":
            load_ring(d, 0)
        if nb > 1:
            for d in "fb":
                load_ring(d, 1)
        for d in "fb":
            for k in range(8):
                jit_piece(d, 0, k)

        for s in range(T):
            blk, sl = divmod(s, SB)
            grp, ts = divmod(s, SJ)
            for d in "fb":
                c = st[d]
                if sl == 0:
                    if blk + 2 < nb:
                        load_ring(d, blk + 2)
                    c["R"] = hpool.tile([H, SB * BC], F16,
                                        tag=f"R{d}", name=f"R{d}")
            # Whh matmuls + one JIT piece of the NEXT group per chain
            for d in "fb":
                c = st[d]
                bank = c["banks"][grp]
                for m in range(4):
                    o = m * NG + ts * BC
                    nc.tensor.matmul(bank[:, o:o + BC],
                                     c["whh"][:, m * 128:(m + 1) * 128],
                                     c["h_prev"], start=False, stop=True,
                                     skip_group_check=True)
                jit_piece(d, grp + 1, ts)
            # gate sigmoid
            for d in "fb":
                c = st[d]
                gview = (c["banks"][grp][0:H, :]
                         .rearrange("p (m t b) -> p m t b", m=4, t=SJ)
                         [:, :, ts, :])
                c["sga"] = spool.tile([H, 4 * BC], F32, tag=f"sga{d}",
                                      name=f"sga{d}")
                nc.scalar.activation(
                    c["sga"][:].rearrange("p (m b) -> p m b", m=4), gview,
                    AF.Sigmoid)
            # cell (all DVE, chain-major so C_f isn't queued behind b's ops)
            for d in "fb":
                c = st[d]
                sga = c["sga"]
                c["t1q"] = spool.tile([H, BC], F32, tag=f"t1q{d}",
                                      name=f"t1q{d}")
                nc.vector.scalar_tensor_tensor(
                    c["t1q"][:], sga[:, 3 * BC:4 * BC], -0.5, sga[:, 0:BC],
                    OP.add, OP.mult)
                c["t2"] = spool.tile([H, BC], F32, tag=f"t2{d}", name=f"t2{d}")
                nc.vector.tensor_tensor(
                    c["t2"][:], sga[:, BC:2 * BC], c["c_prev"], OP.mult)
                Cn = cpool.tile([H, BC], F32, tag=f"C{d}", name=f"C{d}")
                nc.vector.scalar_tensor_tensor(
                    Cn[:], c["t1q"][:], 4.0, c["t2"][:], OP.mult, OP.add)
                c["Cn"] = Cn
            # sigmoid(C)
            for d in "fb":
                c = st[d]
                c["sc"] = spool.tile([H, BC], F32, tag=f"sc{d}", name=f"sc{d}")
                nc.scalar.activation(c["sc"][:], c["Cn"][:], AF.Sigmoid)
            # h = (sc-0.5)*so   (DVE)
            for d in "fb":
                c = st[d]
                hsl = c["R"][:, sl * BC:(sl + 1) * BC]
                nc.vector.scalar_tensor_tensor(
                    hsl, c["sc"][:], -0.5, c["sga"][:, 2 * BC:3 * BC],
                    OP.add, OP.mult)
                c["h_prev"], c["c_prev"] = hsl, c["Cn"][:]
                if sl == SB - 1:
                    rev = (d == "b")
                    pblk = (nb - 1 - blk) if rev else blk
                    dst = c["hout"][:, pblk * SB:(pblk + 1) * SB, :]
                    srcv = c["R"][:].rearrange("p (t b) -> p t b", t=SB)
                    if rev:
                        srcv = srcv[:, ::-1, :]
                    nc.sync.dma_start(dst, srcv)

    for _ in range(REPEAT):
        recurrence(0)
        recurrence(1)
    ctx.close()


# --------------------------------------------------------------------------
# host side
# --------------------------------------------------------------------------

def _prep(w, scale_g=True, scale_all=1.0):
    """w: (400, D) post-perm rows; double g rows, apply overall scale."""
    w = w.copy()
    if scale_g:
        w[300:400] *= 2.0
    return w * scale_all


def make_in_maps(x, w_ih0, w_hh0, b0, w_ih1, w_hh1, b1, T):
    x = np.asarray(x, np.float32)
    shared = {}
    for d, di in (("f", 0), ("b", 1)):
        for lname, whh in (("whh0", w_hh0), ("whh1", w_hh1)):
            w = _prep(np.asarray(whh[di], np.float32)[_PERM], scale_all=2.0)
            wt = w.T.reshape(H, 4, H)
            wp = np.zeros((H, 4, 128), np.float16)
            wp[:, :, :H] = wt.astype(np.float16)
            shared[f"{lname}{d}"] = wp
        def chunkpad(wt, dtype):
            # wt: (rows, 400) -> (rows, 4, 128) zero-padded
            rows = wt.shape[0]
            wp = np.zeros((rows, 4, 128), dtype)
            wp[:, :, :H] = wt.reshape(rows, 4, H).astype(dtype)
            return wp

        bb0 = _prep(np.asarray(b0[di], np.float32)[_PERM][:, None])[:, 0]
        wi0 = _prep(np.asarray(w_ih0[di], np.float32)[_PERM])
        shared[f"wih0{d}"] = chunkpad(
            np.concatenate([wi0.T, bb0[None]], 0), np.float16)
        bb1 = _prep(np.asarray(b1[di], np.float32)[_PERM][:, None])[:, 0]
        wi1 = _prep(np.asarray(w_ih1[di], np.float32)[_PERM], scale_all=2.0)
        shared[f"wih1a{d}"] = chunkpad(wi1[:, :H].T, np.float16)
        shared[f"wih1b{d}"] = chunkpad(
            np.concatenate([wi1[:, H:].T, bb1[None]], 0), np.float16)

    in_maps = []
    for c in range(NCORES):
        xs = x[c * BC:(c + 1) * BC]
        xf = np.ascontiguousarray(xs.transpose(1, 2, 0))
        xe = np.concatenate([xf, np.ones((1, T, BC), np.float32)], 0)
        m = dict(shared)
        m["xe"] = np.ascontiguousarray(xe).astype(np.float16)
        m["xer"] = np.ascontiguousarray(xe[:, ::-1, :]).astype(np.float16)
        in_maps.append(m)
    return in_maps


def assemble_output(results, T):
    out = np.empty((T, NCORES * BC, 2 * H), np.float32)
    for c, r in enumerate(results):
        out[:, c * BC:(c + 1) * BC, :H] = \
            2.0 * r["h1f"].astype(np.float32).transpose(1, 2, 0)
        out[:, c * BC:(c + 1) * BC, H:] = \
            2.0 * r["h1b"].astype(np.float32).transpose(1, 2, 0)
    return out


OUT_SCALE = 2.0
_CACHE = {}
TRACE = False
LAST_RESULTS = None


def _get_program(T=1024):
    if T not in _CACHE:
        nc = build_program(T=T)
        nc.finalize()
        _CACHE[T] = nc
    return _CACHE[T]


def kernel(x, w_ih0, w_hh0, b0, w_ih1, w_hh1, b1):
    global LAST_RESULTS
    T = x.shape[2]
    nc = _get_program(T)
    in_maps = make_in_maps(x, w_ih0, w_hh0, b0, w_ih1, w_hh1, b1, T)
    res = bass_utils.run_bass_kernel_spmd(nc, in_maps,
                                          core_ids=list(range(NCORES)),
                                          trace=TRACE)
    LAST_RESULTS = res
    return assemble_output(res.results, T)

